# revision 1
# baseline (speedup 1.0000x reference)
"""Trainium2 Bass kernel for a Neural ODE (tanh-MLP vector field, Heun/RK2).

Reference computation (per batch row y of width D=512):
    f(y) = tanh(y @ W1 + b1) @ W2 + b2          (H = 2048)
    10 Heun steps, dt = 0.1:
        k1 = f(y); k2 = f(y + dt*k1); y <- y + dt/2*(k1 + k2)

Sharding: data-parallel over the batch axis across 8 NeuronCores
(y0 [8192,512] -> 8 x [1024,512]); weights replicated.

Per-core layout: the state lives TRANSPOSED (y.T, [D, B_local] with D on
partitions) so both matmuls of the MLP chain need no on-chip transposes:
    h.T = W1.T @ y.T   (lhsT = W1 [K=D, M=H],  rhs = y.T  [K=D, N=B])
    z.T = W2.T @ ht.T  (lhsT = W2 [K=H, M=D],  rhs = ht.T [K=H, N=B])
The batch-major <-> feature-major layout conversion is done host-side in
numpy (a few ms on 8+8 MB), so the device runs a pure matmul pipeline.
Matmul operands are stored as float32r (FP22), which streams at 1
cycle/row (full bf16 rate) with fp32 PSUM accumulation.

The batch (N) axis is processed as two 512-wide chunks whose matmuls are
emitted as back-to-back pairs sharing the same stationary weights, and
walrus is run with --enable-ldw-opt=true so the duplicate LDWEIGHTS of
each pair is elided, hiding the weight-load time entirely.
"""

import numpy as np

import concourse.bacc as bacc
import concourse.bass_utils as _bass_utils
import concourse.mybir as mybir
import concourse.tile as tile
from concourse.bass_utils import run_bass_kernel_spmd

# Elide back-to-back LDWEIGHTS of identical weights (our matmul pairs).
if not getattr(_bass_utils, "_ldw_opt_patched", False):
    _orig_run_command = _bass_utils.run_command

    def _run_command_ldw_opt(argv, **kwargs):
        argv = ["--enable-ldw-opt=true" if a == "--enable-ldw-opt=false" else a
                for a in argv]
        return _orig_run_command(argv, **kwargs)

    _bass_utils.run_command = _run_command_ldw_opt
    _bass_utils._ldw_opt_patched = True

N_CORES = 8
BATCH, D, H = 8192, 512, 2048
B = BATCH // N_CORES          # local batch per core: 1024
DT = 0.1
N_STEPS = 10
P = 128
F32 = mybir.dt.float32
F32R = mybir.dt.float32r

D_T = D // P                  # 4  k-tiles / d-tiles
H_T = H // P                  # 16 h-tiles
NCHUNK = 2                    # batch chunks per core (N=512 per matmul)
NW = B // NCHUNK              # 512

_NC_CACHE = None


def _build():
    nc = bacc.Bacc("TRN2", target_bir_lowering=False, debug=False)
    # y0t / outt are the batch shard pre-transposed to [D, B] on the host.
    y0t = nc.dram_tensor("y0t", [D, B], F32, kind="ExternalInput").ap()
    W1 = nc.dram_tensor("W1", [D, H], F32, kind="ExternalInput").ap()
    b1 = nc.dram_tensor("b1", [H], F32, kind="ExternalInput").ap()
    W2 = nc.dram_tensor("W2", [H, D], F32, kind="ExternalInput").ap()
    b2 = nc.dram_tensor("b2", [D], F32, kind="ExternalInput").ap()
    outt = nc.dram_tensor("outt", [D, B], F32, kind="ExternalOutput").ap()

    TANH = mybir.ActivationFunctionType.Tanh
    MULT = mybir.AluOpType.mult
    ADD = mybir.AluOpType.add
    HALF_DT = DT / 2.0

    with tile.TileContext(nc) as tc:
        with (
            tc.tile_pool(name="persist", bufs=1) as persist,
            tc.tile_pool(name="ps_h", bufs=4, space="PSUM") as ps_h_pool,
            tc.tile_pool(name="ps_z", bufs=4, space="PSUM") as ps_z_pool,
        ):
            # Persistent SBUF residents (per-partition bytes in parens).
            w1_k = [persist.tile([P, H], F32R, tag=f"w1k{kt}", name=f"w1k{kt}")
                    for kt in range(D_T)]                        # 32K
            w2_k = [persist.tile([P, D], F32R, tag=f"w2k{kt}", name=f"w2k{kt}")
                    for kt in range(H_T)]                        # 32K
            b1_sb = persist.tile([P, H_T], F32, tag="b1")
            b2_sb = persist.tile([P, D_T], F32, tag="b2")
            y_sb = persist.tile([P, D_T * B], F32R, tag="y")     # 16K
            y_acc = persist.tile([P, D_T * B], F32, tag="yacc")  # 16K
            y_mid = persist.tile([P, D_T * B], F32R, tag="ymid")  # 16K
            ht_sb = persist.tile([P, H_T * B], F32R, tag="ht")   # 64K

            # --- input DMAs, in consumption order ---
            for kt in range(D_T):
                nc.sync.dma_start(y_sb[:, kt * B:(kt + 1) * B],
                                  y0t[kt * P:(kt + 1) * P, :].bitcast(F32R))
            for kt in range(D_T):
                nc.sync.dma_start(w1_k[kt][:],
                                  W1[kt * P:(kt + 1) * P, :].bitcast(F32R))
            nc.sync.dma_start(b1_sb[:], b1.rearrange("(m p) -> p m", p=P))
            nc.sync.dma_start(b2_sb[:], b2.rearrange("(m p) -> p m", p=P))
            for kt in range(H_T):
                nc.sync.dma_start(w2_k[kt][:],
                                  W2[kt * P:(kt + 1) * P, :].bitcast(F32R))

            def feval(X, consume):
                """One vector-field evaluation: z.T = W2.T@tanh(W1.T@X + b1).

                X: SBUF state tile [P, D_T*B] holding X.T; consume(dm, n0, pz)
                receives each z.T output PSUM tile [P, NW] (pre-b2).
                Both batch chunks advance together as weight-sharing matmul
                pairs.
                """
                for m in range(H_T):
                    ph = [ps_h_pool.tile([P, NW], F32, tag="ps_h", name="ph")
                          for _ in range(NCHUNK)]
                    for kt in range(D_T):
                        w_ap = w1_k[kt][:, m * P:(m + 1) * P]
                        for c in range(NCHUNK):
                            nc.tensor.matmul(
                                ph[c][:], w_ap,
                                X[:, kt * B + c * NW: kt * B + c * NW + NW],
                                start=(kt == 0), stop=(kt == D_T - 1))
                    for c in range(NCHUNK):
                        nc.scalar.activation(
                            ht_sb[:, m * B + c * NW: m * B + (c + 1) * NW],
                            ph[c][:], TANH, bias=b1_sb[:, m:m + 1])
                for dm in range(D_T):
                    pz = [ps_z_pool.tile([P, NW], F32, tag="ps_z", name="pz")
                          for _ in range(NCHUNK)]
                    for kt in range(H_T):
                        w_ap = w2_k[kt][:, dm * P:(dm + 1) * P]
                        for c in range(NCHUNK):
                            nc.tensor.matmul(
                                pz[c][:], w_ap,
                                ht_sb[:, kt * B + c * NW: kt * B + c * NW + NW],
                                start=(kt == 0), stop=(kt == H_T - 1))
                    for c in range(NCHUNK):
                        consume(dm, c * NW, pz[c])

            def consume_k1(dm, n0, pz):
                off = dm * B + n0
                # z -> k1 = z + b2 ; y_mid = y + dt*k1 ; y_acc = y + dt/2*k1
                nc.vector.tensor_scalar_add(pz[:], pz[:], b2_sb[:, dm:dm + 1])
                nc.vector.scalar_tensor_tensor(
                    y_mid[:, off:off + NW], pz[:], DT, y_sb[:, off:off + NW],
                    op0=MULT, op1=ADD)
                nc.vector.scalar_tensor_tensor(
                    y_acc[:, off:off + NW], pz[:], HALF_DT, y_sb[:, off:off + NW],
                    op0=MULT, op1=ADD)

            def consume_k2(dm, n0, pz):
                off = dm * B + n0
                # y <- y_acc + dt/2*(z + b2)
                nc.vector.tensor_scalar_add(pz[:], pz[:], b2_sb[:, dm:dm + 1])
                nc.vector.scalar_tensor_tensor(
                    y_sb[:, off:off + NW], pz[:], HALF_DT, y_acc[:, off:off + NW],
                    op0=MULT, op1=ADD)

            for _step in range(N_STEPS):
                feval(y_sb, consume_k1)
                feval(y_mid, consume_k2)

            # --- final store: y.T tiles straight out; host re-transposes ---
            for kt in range(D_T):
                nc.sync.dma_start(outt[kt * P:(kt + 1) * P, :],
                                  y_sb[:, kt * B:(kt + 1) * B].bitcast(F32))

    nc.compile()
    return nc


def get_nc():
    global _NC_CACHE
    if _NC_CACHE is None:
        _NC_CACHE = _build()
    return _NC_CACHE


def run(inputs, trace=False, **kwargs):
    nc = get_nc()
    y0 = np.asarray(inputs["y0"], dtype=np.float32)
    W1 = np.ascontiguousarray(np.asarray(inputs["W1"], dtype=np.float32))
    b1 = np.ascontiguousarray(np.asarray(inputs["b1"], dtype=np.float32))
    W2 = np.ascontiguousarray(np.asarray(inputs["W2"], dtype=np.float32))
    b2 = np.ascontiguousarray(np.asarray(inputs["b2"], dtype=np.float32))
    # shard over batch, pre-transpose each shard to [D, B] feature-major
    shards_t = np.ascontiguousarray(
        y0.reshape(N_CORES, B, D).transpose(0, 2, 1))
    in_maps = [{"y0t": shards_t[i], "W1": W1, "b1": b1, "W2": W2, "b2": b2}
               for i in range(N_CORES)]
    res = run_bass_kernel_spmd(nc, in_maps, core_ids=list(range(N_CORES)),
                               trace=trace, **kwargs)
    out_t = np.stack([r["outt"] for r in res.results])      # [8, D, B]
    full = np.ascontiguousarray(
        out_t.transpose(0, 2, 1).reshape(BATCH, D))
    return full, res


def kernel(**inputs) -> np.ndarray:
    full, _ = run(inputs, trace=False)
    return full



# revision 4
# speedup vs baseline: 1.7533x; 1.7533x over previous
"""Trainium2 Bass kernel for a Neural ODE (tanh-MLP vector field, Heun/RK2).

Reference computation (per batch row y of width D=512):
    f(y) = tanh(y @ W1 + b1) @ W2 + b2          (H = 2048)
    10 Heun steps, dt = 0.1:
        k1 = f(y); k2 = f(y + dt*k1); y <- y + dt/2*(k1 + k2)

Sharding: data-parallel over the batch axis across 8 NeuronCores
(y0 [8192,512] -> 8 x [1024,512]); weights replicated.

Per-core layout: the state lives TRANSPOSED (y.T, [D, B_local] with D on
partitions) so both matmuls of the MLP chain need no on-chip transposes:
    h.T = W1.T @ y.T   (lhsT = W1 [K=D, M=H],  rhs = y.T  [K=D, N=B])
    z.T = W2.T @ ht.T  (lhsT = W2 [K=H, M=D],  rhs = ht.T [K=H, N=B])

All matmuls run in fp8-e4m3 with perf_mode=DoubleRow (contraction 256 per
pass, ~1.8x the bf16/f32r rate). Precision is recovered two ways:
  1. The ODE state Y = SY*y and the Heun updates stay fp32; only the
     matmul operands (Y8, Ymid8, tanh output) are fp8, with power-of-two
     tensor scales (SY, S1, S2) folded into activation scale / update
     scalars so quantization sits in e4m3's sweet spot.
  2. The dominant error source -- the *systematic* weight quantization
     error integrated over all 20 vector-field evals -- is suppressed by
     keeping KW=4 residual-compensated fp8 copies of each weight matrix
     (copy i rounds (i+1)*W - sum(copies[:i]), so their running mean
     tracks W to ~1/KW of one rounding error) and rotating through them
     across evals.  Measured end-to-end rel-l2 vs the fp32 reference:
     8.8e-3 (vs 2.3e-2 with a single RNE fp8 copy).

Batch chunks are paired inside 2-bank PSUM tiles so ScalarE/DVE post-ops
process 1024 elements per instruction; walrus ldw-opt is NOT used (it
rejects DoubleRow LDWEIGHTS) -- duplicate weight loads hide under the
matmul stream via the PE background weight buffer.
"""

import numpy as np
import ml_dtypes

import concourse.bacc as bacc
import concourse.mybir as mybir
import concourse.tile as tile
from concourse.bass_utils import run_bass_kernel_spmd

N_CORES = 8
BATCH, D, H = 8192, 512, 2048
B = BATCH // N_CORES          # local batch per core: 1024
DT = 0.1
N_STEPS = 10
P = 128
F32 = mybir.dt.float32
F8 = mybir.dt.float8e4
U8 = mybir.dt.uint8

D_T = D // P                  # 4  d-tiles (128-row feature blocks)
H_T = H // P                  # 16 h-tiles
KT1 = D // 256                # 2  DoubleRow k-tiles, layer 1
KT2 = H // 256                # 8  DoubleRow k-tiles, layer 2
NCHUNK = 2                    # batch chunks per core (N=512 per matmul)
NW = B // NCHUNK              # 512

KW = 4                        # rotated fp8 weight copies
SY = 16.0                     # state scale   (Y = SY*y)
S1 = 256.0                    # W1 scale
S2 = 1024.0                   # W2 scale
DR = mybir.MatmulPerfMode.DoubleRow

_NC_CACHE = {}


def _build(has_b2):
    nc = bacc.Bacc("TRN2", target_bir_lowering=False, debug=False)
    # Host-side prep: y0t = SY * y0_shard.T (fp32); y0q = e4m3(SY*y0.T) bits;
    # w1q/w2q = KW residual-compensated e4m3 copies of S1*W1 / S2*W2 (bits);
    # b2s = S2*b2.
    y0t = nc.dram_tensor("y0t", [D, B], F32, kind="ExternalInput").ap()
    y0q = nc.dram_tensor("y0q", [D, B], U8, kind="ExternalInput").ap()
    w1q = nc.dram_tensor("w1q", [KW, D, H], U8, kind="ExternalInput").ap()
    b1 = nc.dram_tensor("b1", [H], F32, kind="ExternalInput").ap()
    w2q = nc.dram_tensor("w2q", [KW, H, D], U8, kind="ExternalInput").ap()
    b2s = nc.dram_tensor("b2s", [D], F32, kind="ExternalInput").ap()
    outt = nc.dram_tensor("outt", [D, B], F32, kind="ExternalOutput").ap()

    TANH = mybir.ActivationFunctionType.Tanh
    MULT = mybir.AluOpType.mult
    ADD = mybir.AluOpType.add
    ASCALE = 1.0 / (S1 * SY)         # PSUM1 -> pre-activation
    C1 = DT * SY / S2                # Ymid = Y + C1*pzb
    CH = 0.5 * DT * SY / S2          # Yacc/Ynew = ... + CH*pzb

    with tile.TileContext(nc) as tc:
        with (
            tc.tile_pool(name="persist", bufs=1) as persist,
            tc.tile_pool(name="ps_h", bufs=3, space="PSUM") as ps_h_pool,
            tc.tile_pool(name="ps_z", bufs=1, space="PSUM") as ps_z_pool,
        ):
            # Persistent SBUF residents (per-partition bytes in parens).
            w1_sb = [[persist.tile([P, 2, H], F8, tag=f"w1_{i}_{t}",
                                   name=f"w1_{i}_{t}")
                      for t in range(KT1)] for i in range(KW)]   # 4K x 8
            w2_sb = [[persist.tile([P, 2, D], F8, tag=f"w2_{i}_{t}",
                                   name=f"w2_{i}_{t}")
                      for t in range(KT2)] for i in range(KW)]   # 1K x 32
            b1_sb = persist.tile([P, H_T], F32, tag="b1")
            b2_sb = persist.tile([P, D_T], F32, tag="b2")
            y_sb = persist.tile([P, D_T, NCHUNK, NW], F32, tag="y")      # 16K
            y_acc = persist.tile([P, D_T, NCHUNK, NW], F32, tag="yacc")  # 16K
            y8 = persist.tile([P, D_T, NCHUNK, NW], F8, tag="y8")        # 4K
            ym8 = persist.tile([P, D_T, NCHUNK, NW], F8, tag="ym8")      # 4K
            ht8 = persist.tile([P, H_T, NCHUNK, NW], F8, tag="ht8")      # 16K

            # --- input DMAs, in consumption order ---
            for kt in range(D_T):
                nc.sync.dma_start(y8[:, kt, :, :],
                                  y0q[kt * P:(kt + 1) * P, :].bitcast(F8))
            for t in range(KT1):
                for o in range(2):
                    r = (2 * t + o) * P
                    nc.sync.dma_start(w1_sb[0][t][:, o, :],
                                      w1q[0, r:r + P, :].bitcast(F8))
            nc.sync.dma_start(b1_sb[:], b1.rearrange("(m p) -> p m", p=P))
            nc.sync.dma_start(b2_sb[:], b2s.rearrange("(m p) -> p m", p=P))
            for t in range(KT2):
                for o in range(2):
                    r = (2 * t + o) * P
                    nc.sync.dma_start(w2_sb[0][t][:, o, :],
                                      w2q[0, r:r + P, :].bitcast(F8))
            for kt in range(D_T):
                nc.sync.dma_start(y_sb[:, kt, :, :],
                                  y0t[kt * P:(kt + 1) * P, :])
            for i in range(1, KW):
                for t in range(KT1):
                    for o in range(2):
                        r = (2 * t + o) * P
                        nc.sync.dma_start(w1_sb[i][t][:, o, :],
                                          w1q[i, r:r + P, :].bitcast(F8))
                for t in range(KT2):
                    for o in range(2):
                        r = (2 * t + o) * P
                        nc.sync.dma_start(w2_sb[i][t][:, o, :],
                                          w2q[i, r:r + P, :].bitcast(F8))

            def feval(X8, wi, consume):
                """One vector-field evaluation on fp8 state X8 [P,D_T,2,NW].

                Layer 1: psum = W1q.T @ X8 (DoubleRow, K=256/pass);
                ht8 = e4m3(tanh(psum/(S1*SY) + b1)), one 1024-elem ACT per
                m covering both batch chunks (2-bank PSUM tile).
                Layer 2: pz = W2q.T @ ht8; consume(dm, pz2) handles the
                S2-scaled vector field [P, 2, NW].
                """
                for m in range(H_T):
                    ph = ps_h_pool.tile([P, NCHUNK, NW], F32, tag="ps_h",
                                        name="ph")
                    for t in range(KT1):
                        w_ap = w1_sb[wi][t][:, :, m * P:(m + 1) * P]
                        for c in range(NCHUNK):
                            nc.tensor.matmul(
                                ph[:, c, :], w_ap,
                                X8[:, 2 * t:2 * t + 2, c, :],
                                start=(t == 0), stop=(t == KT1 - 1),
                                perf_mode=DR)
                    nc.scalar.activation(
                        ht8[:, m, :, :], ph[:, :, :], TANH,
                        bias=b1_sb[:, m:m + 1], scale=ASCALE)
                for dm in range(D_T):
                    pz = ps_z_pool.tile([P, NCHUNK, NW], F32, tag="ps_z",
                                        name="pz")
                    for t in range(KT2):
                        w_ap = w2_sb[wi][t][:, :, dm * P:(dm + 1) * P]
                        for c in range(NCHUNK):
                            nc.tensor.matmul(
                                pz[:, c, :], w_ap,
                                ht8[:, 2 * t:2 * t + 2, c, :],
                                start=(t == 0), stop=(t == KT2 - 1),
                                perf_mode=DR)
                    consume(dm, pz)

            def consume_k1(dm, pz):
                # pzb = pz (+ S2*b2) = S2*f(y);  Ymid8 = e4m3(Y + C1*pzb);
                # Yacc = Y + CH*pzb.
                if has_b2:
                    nc.vector.tensor_scalar_add(pz[:, :, :], pz[:, :, :],
                                                b2_sb[:, dm:dm + 1])
                nc.vector.scalar_tensor_tensor(
                    ym8[:, dm, :, :], pz[:, :, :], C1, y_sb[:, dm, :, :],
                    op0=MULT, op1=ADD)
                nc.vector.scalar_tensor_tensor(
                    y_acc[:, dm, :, :], pz[:, :, :], CH, y_sb[:, dm, :, :],
                    op0=MULT, op1=ADD)

            def make_consume_k2(last):
                def consume_k2(dm, pz):
                    # Y8' = e4m3(Yacc + CH*pzb) first (unblocks the next
                    # step's layer 1), then the fp32 state write.
                    if has_b2:
                        nc.vector.tensor_scalar_add(pz[:, :, :], pz[:, :, :],
                                                    b2_sb[:, dm:dm + 1])
                    if not last:
                        nc.vector.scalar_tensor_tensor(
                            y8[:, dm, :, :], pz[:, :, :], CH,
                            y_acc[:, dm, :, :], op0=MULT, op1=ADD)
                    nc.vector.scalar_tensor_tensor(
                        y_sb[:, dm, :, :], pz[:, :, :], CH,
                        y_acc[:, dm, :, :], op0=MULT, op1=ADD)
                return consume_k2

            for step in range(N_STEPS):
                feval(y8, (2 * step) % KW, consume_k1)
                feval(ym8, (2 * step + 1) % KW,
                      make_consume_k2(step == N_STEPS - 1))

            # --- final store: Y.T tiles out; host re-transposes & /SY ---
            for kt in range(D_T):
                nc.sync.dma_start(outt[kt * P:(kt + 1) * P, :],
                                  y_sb[:, kt, :, :])

    nc.compile()
    return nc


def get_nc(has_b2=False):
    if has_b2 not in _NC_CACHE:
        _NC_CACHE[has_b2] = _build(has_b2)
    return _NC_CACHE[has_b2]


def _comp_copies(W, s):
    """KW residual-compensated e4m3 copies of s*W, as uint8 bit patterns."""
    sW = (s * W).astype(np.float32)
    copies, acc = [], np.zeros_like(sW)
    for i in range(KW):
        c = np.clip((i + 1) * sW - acc, -240.0, 240.0) \
            .astype(ml_dtypes.float8_e4m3)
        copies.append(c.view(np.uint8))
        acc += c.astype(np.float32)
    return np.ascontiguousarray(np.stack(copies))


def run(inputs, trace=False, **kwargs):
    y0 = np.asarray(inputs["y0"], dtype=np.float32)
    W1 = np.ascontiguousarray(np.asarray(inputs["W1"], dtype=np.float32))
    b1 = np.ascontiguousarray(np.asarray(inputs["b1"], dtype=np.float32))
    W2 = np.ascontiguousarray(np.asarray(inputs["W2"], dtype=np.float32))
    b2 = np.ascontiguousarray(np.asarray(inputs["b2"], dtype=np.float32))
    nc = get_nc(has_b2=bool(np.any(b2)))
    w1q = _comp_copies(W1, S1)
    w2q = _comp_copies(W2, S2)
    b2s = np.ascontiguousarray(np.float32(S2) * b2)
    # shard over batch, pre-transpose each shard to [D, B] feature-major,
    # pre-scale by SY; plus the e4m3 bits of the scaled shard.
    shards_t = np.ascontiguousarray(
        (np.float32(SY) * y0).reshape(N_CORES, B, D).transpose(0, 2, 1))
    shards_q = np.ascontiguousarray(
        np.clip(shards_t, -240.0, 240.0).astype(ml_dtypes.float8_e4m3)
        .view(np.uint8))
    in_maps = [{"y0t": shards_t[i], "y0q": shards_q[i],
                "w1q": w1q, "b1": b1, "w2q": w2q, "b2s": b2s}
               for i in range(N_CORES)]
    res = run_bass_kernel_spmd(nc, in_maps, core_ids=list(range(N_CORES)),
                               trace=trace, **kwargs)
    out_t = np.stack([r["outt"] for r in res.results])      # [8, D, B]
    full = np.ascontiguousarray(
        out_t.transpose(0, 2, 1).reshape(BATCH, D) * np.float32(1.0 / SY))
    return full, res


def kernel(**inputs) -> np.ndarray:
    full, _ = run(inputs, trace=False)
    return full


# revision 5
# speedup vs baseline: 2.0759x; 1.1840x over previous
"""Trainium2 Bass kernel for a Neural ODE (tanh-MLP vector field, Heun/RK2).

Reference computation (per batch row y of width D=512):
    f(y) = tanh(y @ W1 + b1) @ W2 + b2          (H = 2048)
    10 Heun steps, dt = 0.1:
        k1 = f(y); k2 = f(y + dt*k1); y <- y + dt/2*(k1 + k2)

Sharding: data-parallel over the batch axis across 8 NeuronCores
(y0 [8192,512] -> 8 x [1024,512]); weights replicated.

Per-core layout: the state lives TRANSPOSED (y.T, [D, B_local] with D on
partitions) so both matmuls of the MLP chain need no on-chip transposes:
    h.T = W1.T @ y.T   (lhsT = W1 [K=D, M=H],  rhs = y.T  [K=D, N=B])
    z.T = W2.T @ ht.T  (lhsT = W2 [K=H, M=D],  rhs = ht.T [K=H, N=B])

All matmuls run in fp8-e4m3 with perf_mode=DoubleRow (contraction 256 per
pass, ~1.8x the bf16/f32r rate). Precision is recovered two ways:
  1. The ODE state Y = SY*y and the Heun updates stay fp32; only the
     matmul operands (Y8, Ymid8, tanh output) are fp8, with power-of-two
     tensor scales (SY, S1, S2) folded into activation scale / update
     scalars so quantization sits in e4m3's sweet spot.
  2. The dominant error source -- the *systematic* weight quantization
     error integrated over all 20 vector-field evals -- is suppressed by
     keeping KW=4 residual-compensated fp8 copies of each weight matrix
     (copy i rounds (i+1)*W - sum(copies[:i]), so their running mean
     tracks W to ~1/KW of one rounding error) and rotating through them
     across evals.  Measured end-to-end rel-l2 vs the fp32 reference:
     8.8e-3 (vs 2.3e-2 with a single RNE fp8 copy).

Batch chunks are paired inside 2-bank PSUM tiles so ScalarE/DVE post-ops
process 1024 elements per instruction; walrus ldw-opt is NOT used (it
rejects DoubleRow LDWEIGHTS) -- duplicate weight loads hide under the
matmul stream via the PE background weight buffer.
"""

import numpy as np
import ml_dtypes

import concourse.bacc as bacc
import concourse.mybir as mybir
import concourse.tile as tile
from concourse.bass_utils import run_bass_kernel_spmd

N_CORES = 8
BATCH, D, H = 8192, 512, 2048
B = BATCH // N_CORES          # local batch per core: 1024
DT = 0.1
N_STEPS = 10
P = 128
F32 = mybir.dt.float32
F8 = mybir.dt.float8e4
U8 = mybir.dt.uint8

D_T = D // P                  # 4  d-tiles (128-row feature blocks)
H_T = H // P                  # 16 h-tiles
KT1 = D // 256                # 2  DoubleRow k-tiles, layer 1
KT2 = H // 256                # 8  DoubleRow k-tiles, layer 2
NCHUNK = 2                    # batch chunks per core (N=512 per matmul)
NW = B // NCHUNK              # 512

KW = 4                        # rotated fp8 weight copies
SY = 16.0                     # state scale   (Y = SY*y)
S1 = 256.0                    # W1 scale
S2 = 1024.0                   # W2 scale
DR = mybir.MatmulPerfMode.DoubleRow

_NC_CACHE = {}


def _build(has_b2):
    nc = bacc.Bacc("TRN2", target_bir_lowering=False, debug=False)
    # Host-side prep: y0t = SY * y0_shard.T (fp32); y0q = e4m3(SY*y0.T) bits;
    # w1q/w2q = KW residual-compensated e4m3 copies of S1*W1 / S2*W2 (bits);
    # b2s = S2*b2.
    y0t = nc.dram_tensor("y0t", [D, B], F32, kind="ExternalInput").ap()
    y0q = nc.dram_tensor("y0q", [D, B], U8, kind="ExternalInput").ap()
    w1q = nc.dram_tensor("w1q", [KW, D, H], U8, kind="ExternalInput").ap()
    b1 = nc.dram_tensor("b1", [H], F32, kind="ExternalInput").ap()
    w2q = nc.dram_tensor("w2q", [KW, H, D], U8, kind="ExternalInput").ap()
    b2s = nc.dram_tensor("b2s", [D], F32, kind="ExternalInput").ap()
    outt = nc.dram_tensor("outt", [D, B], F32, kind="ExternalOutput").ap()

    TANH = mybir.ActivationFunctionType.Tanh
    MULT = mybir.AluOpType.mult
    ADD = mybir.AluOpType.add
    ASCALE = 1.0 / (S1 * SY)         # PSUM1 -> pre-activation
    C1 = DT * SY / S2                # Ymid = Y + C1*pzb
    CH = 0.5 * DT * SY / S2          # Yacc/Ynew = ... + CH*pzb

    with tile.TileContext(nc) as tc:
        with (
            tc.tile_pool(name="persist", bufs=1) as persist,
            tc.tile_pool(name="ps_h", bufs=2, space="PSUM") as ps_h_pool,
            tc.tile_pool(name="ps_z", bufs=2, space="PSUM") as ps_z_pool,
        ):
            # Persistent SBUF residents (per-partition bytes in parens).
            w1_sb = [[persist.tile([P, 2, H], F8, tag=f"w1_{i}_{t}",
                                   name=f"w1_{i}_{t}")
                      for t in range(KT1)] for i in range(KW)]   # 4K x 8
            w2_sb = [[persist.tile([P, 2, D], F8, tag=f"w2_{i}_{t}",
                                   name=f"w2_{i}_{t}")
                      for t in range(KT2)] for i in range(KW)]   # 1K x 32
            b1_sb = persist.tile([P, H_T], F32, tag="b1")
            b2_sb = persist.tile([P, D_T], F32, tag="b2")
            y_sb = persist.tile([P, D_T, NCHUNK, NW], F32, tag="y")      # 16K
            y_acc = persist.tile([P, D_T, NCHUNK, NW], F32, tag="yacc")  # 16K
            y8 = persist.tile([P, D_T, NCHUNK, NW], F8, tag="y8")        # 4K
            ym8 = persist.tile([P, D_T, NCHUNK, NW], F8, tag="ym8")      # 4K
            ht8 = persist.tile([P, H_T, NCHUNK, NW], F8, tag="ht8")      # 16K

            # --- input DMAs, in consumption order ---
            for kt in range(D_T):
                nc.sync.dma_start(y8[:, kt, :, :],
                                  y0q[kt * P:(kt + 1) * P, :].bitcast(F8))
            for t in range(KT1):
                for o in range(2):
                    r = (2 * t + o) * P
                    nc.sync.dma_start(w1_sb[0][t][:, o, :],
                                      w1q[0, r:r + P, :].bitcast(F8))
            nc.sync.dma_start(b1_sb[:], b1.rearrange("(m p) -> p m", p=P))
            nc.sync.dma_start(b2_sb[:], b2s.rearrange("(m p) -> p m", p=P))
            for t in range(KT2):
                for o in range(2):
                    r = (2 * t + o) * P
                    nc.sync.dma_start(w2_sb[0][t][:, o, :],
                                      w2q[0, r:r + P, :].bitcast(F8))
            for kt in range(D_T):
                nc.sync.dma_start(y_sb[:, kt, :, :],
                                  y0t[kt * P:(kt + 1) * P, :])
            for i in range(1, KW):
                for t in range(KT1):
                    for o in range(2):
                        r = (2 * t + o) * P
                        nc.sync.dma_start(w1_sb[i][t][:, o, :],
                                          w1q[i, r:r + P, :].bitcast(F8))
                for t in range(KT2):
                    for o in range(2):
                        r = (2 * t + o) * P
                        nc.sync.dma_start(w2_sb[i][t][:, o, :],
                                          w2q[i, r:r + P, :].bitcast(F8))

            def feval(X8, wi, consume):
                """One vector-field evaluation on fp8 state X8 [P,D_T,2,NW].

                Layer 1: psum = W1q.T @ X8 (DoubleRow, K=256/pass);
                ht8 = e4m3(tanh(psum/(S1*SY) + b1)), one 1024-elem ACT per
                m covering both batch chunks (2-bank PSUM tile).
                Layer 2: pz = W2q.T @ ht8; consume(dm, pz2) handles the
                S2-scaled vector field [P, 2, NW].
                """
                for m in range(H_T):
                    ph = ps_h_pool.tile([P, NCHUNK, NW], F32, tag="ps_h",
                                        name="ph")
                    for t in range(KT1):
                        w_ap = w1_sb[wi][t][:, :, m * P:(m + 1) * P]
                        for c in range(NCHUNK):
                            nc.tensor.matmul(
                                ph[:, c, :], w_ap,
                                X8[:, 2 * t:2 * t + 2, c, :],
                                start=(t == 0), stop=(t == KT1 - 1),
                                perf_mode=DR)
                    nc.scalar.activation(
                        ht8[:, m, :, :], ph[:, :, :], TANH,
                        bias=b1_sb[:, m:m + 1], scale=ASCALE)
                for dm in range(D_T):
                    pz = ps_z_pool.tile([P, NCHUNK, NW], F32, tag="ps_z",
                                        name="pz")
                    for t in range(KT2):
                        w_ap = w2_sb[wi][t][:, :, dm * P:(dm + 1) * P]
                        for c in range(NCHUNK):
                            nc.tensor.matmul(
                                pz[:, c, :], w_ap,
                                ht8[:, 2 * t:2 * t + 2, c, :],
                                start=(t == 0), stop=(t == KT2 - 1),
                                perf_mode=DR)
                    consume(dm, pz)

            def consume_k1(dm, pz):
                # pzb = pz (+ S2*b2) = S2*f(y);  Ymid8 = e4m3(Y + C1*pzb);
                # Yacc = Y + CH*pzb.
                if has_b2:
                    nc.vector.tensor_scalar_add(pz[:, :, :], pz[:, :, :],
                                                b2_sb[:, dm:dm + 1])
                nc.vector.scalar_tensor_tensor(
                    ym8[:, dm, :, :], pz[:, :, :], C1, y_sb[:, dm, :, :],
                    op0=MULT, op1=ADD)
                nc.vector.scalar_tensor_tensor(
                    y_acc[:, dm, :, :], pz[:, :, :], CH, y_sb[:, dm, :, :],
                    op0=MULT, op1=ADD)

            def make_consume_k2(last):
                def consume_k2(dm, pz):
                    # Y8' = e4m3(Yacc + CH*pzb) first (unblocks the next
                    # step's layer 1), then the fp32 state write.
                    if has_b2:
                        nc.vector.tensor_scalar_add(pz[:, :, :], pz[:, :, :],
                                                    b2_sb[:, dm:dm + 1])
                    if not last:
                        nc.vector.scalar_tensor_tensor(
                            y8[:, dm, :, :], pz[:, :, :], CH,
                            y_acc[:, dm, :, :], op0=MULT, op1=ADD)
                    nc.vector.scalar_tensor_tensor(
                        y_sb[:, dm, :, :], pz[:, :, :], CH,
                        y_acc[:, dm, :, :], op0=MULT, op1=ADD)
                return consume_k2

            for step in range(N_STEPS):
                feval(y8, (2 * step) % KW, consume_k1)
                feval(ym8, (2 * step + 1) % KW,
                      make_consume_k2(step == N_STEPS - 1))

            # --- final store: Y.T tiles out; host re-transposes & /SY ---
            for kt in range(D_T):
                nc.sync.dma_start(outt[kt * P:(kt + 1) * P, :],
                                  y_sb[:, kt, :, :])

    nc.compile()
    return nc


def get_nc(has_b2=False):
    if has_b2 not in _NC_CACHE:
        _NC_CACHE[has_b2] = _build(has_b2)
    return _NC_CACHE[has_b2]


def _comp_copies(W, s):
    """KW residual-compensated e4m3 copies of s*W, as uint8 bit patterns."""
    sW = (s * W).astype(np.float32)
    copies, acc = [], np.zeros_like(sW)
    for i in range(KW):
        c = np.clip((i + 1) * sW - acc, -240.0, 240.0) \
            .astype(ml_dtypes.float8_e4m3)
        copies.append(c.view(np.uint8))
        acc += c.astype(np.float32)
    return np.ascontiguousarray(np.stack(copies))


def run(inputs, trace=False, **kwargs):
    y0 = np.asarray(inputs["y0"], dtype=np.float32)
    W1 = np.ascontiguousarray(np.asarray(inputs["W1"], dtype=np.float32))
    b1 = np.ascontiguousarray(np.asarray(inputs["b1"], dtype=np.float32))
    W2 = np.ascontiguousarray(np.asarray(inputs["W2"], dtype=np.float32))
    b2 = np.ascontiguousarray(np.asarray(inputs["b2"], dtype=np.float32))
    nc = get_nc(has_b2=bool(np.any(b2)))
    w1q = _comp_copies(W1, S1)
    w2q = _comp_copies(W2, S2)
    b2s = np.ascontiguousarray(np.float32(S2) * b2)
    # shard over batch, pre-transpose each shard to [D, B] feature-major,
    # pre-scale by SY; plus the e4m3 bits of the scaled shard.
    shards_t = np.ascontiguousarray(
        (np.float32(SY) * y0).reshape(N_CORES, B, D).transpose(0, 2, 1))
    shards_q = np.ascontiguousarray(
        np.clip(shards_t, -240.0, 240.0).astype(ml_dtypes.float8_e4m3)
        .view(np.uint8))
    in_maps = [{"y0t": shards_t[i], "y0q": shards_q[i],
                "w1q": w1q, "b1": b1, "w2q": w2q, "b2s": b2s}
               for i in range(N_CORES)]
    res = run_bass_kernel_spmd(nc, in_maps, core_ids=list(range(N_CORES)),
                               trace=trace, **kwargs)
    out_t = np.stack([r["outt"] for r in res.results])      # [8, D, B]
    full = np.ascontiguousarray(
        out_t.transpose(0, 2, 1).reshape(BATCH, D) * np.float32(1.0 / SY))
    return full, res


def kernel(**inputs) -> np.ndarray:
    full, _ = run(inputs, trace=False)
    return full
